# revision 20
# baseline (speedup 1.0000x reference)
"""Trainium2 Bass kernel for nn_Attention_78786880078278.

Dense causal multi-head attention layer (QKV proj + RoPE + causal softmax
attention + output proj), sharded over 8 NeuronCores:
  - NEFF 1 (head-parallel): each core computes QKV projections, RoPE and
    causal attention for its 2 heads (x 2 batches), producing per-head
    attention outputs.
  - host: pure relayout (gather + transpose + dtype staging) of tensors.
  - NEFF 2 (token-parallel): each core computes the output projection for
    its (token-half x hout-quarter) block.

All matmuls run in bf16 with fp32 PSUM accumulation. Operands are staged
to bf16 on the host (same RNE rounding the device cast would apply), in
layouts that give every DMA long contiguous partition lines.
"""

import contextlib
import ctypes
import hashlib
import json
import math
import os
import shutil
import sys
import types

import numpy as np

# ---------------------------------------------------------------------------
# environment fixups
# ---------------------------------------------------------------------------

for _p in ("/opt/trn_rl_repo",):
    if _p not in sys.path and os.path.isdir(_p):
        sys.path.append(_p)

import concourse.bass as bass  # noqa: E402
import concourse.bass2jax as bass2jax  # noqa: E402
import concourse.mybir as mybir  # noqa: E402
import concourse.tile as tile  # noqa: E402
from concourse.bass_utils import run_bass_kernel_spmd  # noqa: E402

import ml_dtypes  # noqa: E402

F32 = mybir.dt.float32
BF16 = mybir.dt.bfloat16
NP_BF16 = ml_dtypes.bfloat16

_NEFF_CACHE_DIR = os.environ.get("NEFF_CACHE_DIR", "/tmp/neff_cache")


def _install_compile_fixups():
    """(1) Split multi-wait instructions: this walrus build encodes a single
    sync-wait slot per instruction and rejects Tile's final multi-wait drain.
    (2) Cache compiled NEFFs by BIR hash so repeated runs skip walrus."""
    if getattr(bass2jax, "_attn_fixup_installed", False):
        return
    orig = bass2jax.compile_bir_kernel

    def _fix_multiwait(bir_bytes):
        bir = json.loads(bir_bytes)
        changed = False
        for fn in bir.get("functions", []):
            for blk in fn.get("basic_blocks", fn.get("blocks", [])):
                new_insts = []
                for inst in blk.get("instructions", []):
                    si = inst.get("sync_info") or {}
                    waits = si.get("on_wait") or []
                    if len(waits) > 1:
                        changed = True
                        for i, w in enumerate(waits[:-1]):
                            pre = {
                                "name": f"{inst['name']}_w{i}",
                                "opcode": "Drain",
                                "engine": inst["engine"],
                                "ins": [],
                                "outs": [],
                                "sync_info": {"on_wait": [w], "on_update": []},
                            }
                            if "debug" in inst:
                                pre["debug"] = inst["debug"]
                            if "is_reset_sema" in inst:
                                pre["is_reset_sema"] = False
                            new_insts.append(pre)
                        si["on_wait"] = [waits[-1]]
                        inst["sync_info"] = si
                    new_insts.append(inst)
                blk["instructions"] = new_insts
        return json.dumps(bir).encode() if changed else bir_bytes

    def _patched(bir_json, tmpdir, neff_name="file.neff"):
        fixed = _fix_multiwait(bir_json)
        key = hashlib.sha256(fixed).hexdigest()[:24]
        cached = os.path.join(_NEFF_CACHE_DIR, f"{key}.neff")
        target = os.path.join(tmpdir, neff_name)
        if os.path.exists(cached):
            shutil.copy(cached, target)
            return target
        path = orig(fixed, tmpdir, neff_name)
        try:
            os.makedirs(_NEFF_CACHE_DIR, exist_ok=True)
            shutil.copy(path, cached)
        except OSError:
            pass
        return path

    bass2jax.compile_bir_kernel = _patched
    bass2jax._attn_fixup_installed = True


def _install_ntff_hook():
    """Register the NTFF profiling hook (used only when BASS_TRACE=1)."""
    try:
        import antenv
    except ImportError:
        return
    if "antenv.axon_hooks" in sys.modules:
        return
    so_path = "/opt/axon/libaxon_pjrt.so"
    try:
        lib = ctypes.CDLL(so_path)
    except OSError:
        return
    if not hasattr(lib, "axon_start_nrt_profile"):
        return
    lib.axon_start_nrt_profile.argtypes = [
        ctypes.POINTER(ctypes.c_int64),
        ctypes.c_size_t,
    ]
    lib.axon_start_nrt_profile.restype = ctypes.c_int64
    lib.axon_stop_nrt_profile.argtypes = [ctypes.c_char_p]
    lib.axon_stop_nrt_profile.restype = ctypes.c_int64

    @contextlib.contextmanager
    def _hook(output_dir, device_ids):
        import jax

        jax.devices()
        if device_ids:
            ids = (ctypes.c_int64 * len(device_ids))(*device_ids)
            rc = lib.axon_start_nrt_profile(ids, len(device_ids))
        else:
            rc = lib.axon_start_nrt_profile(None, 0)
        if rc != 0:
            raise RuntimeError(f"axon_start_nrt_profile rc={rc}")
        try:
            yield
        finally:
            n = lib.axon_stop_nrt_profile(str(output_dir).encode())
            print(f"profile: {n} file(s) in {output_dir}", file=sys.stderr)

    mod = types.ModuleType("antenv.axon_hooks")
    mod.get_axon_ntff_profile_hook = lambda: _hook
    mod.set_axon_ntff_profile_hook = lambda h: None
    sys.modules["antenv.axon_hooks"] = mod
    antenv.axon_hooks = mod


_install_compile_fixups()
_install_ntff_hook()

# ---------------------------------------------------------------------------
# problem constants (hardcoded per the task spec)
# ---------------------------------------------------------------------------

HIDDEN = 2048
HEADS = 16
HD = 128  # head dim
B = 2
S = 2048
N_CORES = 8
HPC = HEADS // N_CORES  # heads per core = 2
SPAN = 512
NSPANS = S // SPAN  # 4 query spans per batch
KT = HIDDEN // 128  # 16 contraction tiles
TT = S // 128  # 16 token tiles per batch
SCALE = 1.0 / math.sqrt(HD)
T_TILES = 16  # token tiles per core in NEFF 2 (2048 tokens)
HOUT = HIDDEN // 4  # 512 output channels per core in NEFF 2

LAST_RESULTS = []  # BassKernelResults of the most recent kernel() call


# ---------------------------------------------------------------------------
# NEFF 1: QKV projections + RoPE + causal attention for 2 heads x 2 batches
# ---------------------------------------------------------------------------

def build_attn_nc():
    nc = bass.Bass(target_bir_lowering=False, debug=False)

    # host-tiled bf16 inputs: long contiguous per-partition DMA lines
    xd = nc.dram_tensor("x_t", [B, NSPANS, 128, KT, SPAN], BF16,
                        kind="ExternalInput")
    wqd = nc.dram_tensor("wq_t", [128, KT, HPC * HD], BF16,
                         kind="ExternalInput")
    wkd = nc.dram_tensor("wk_t", [128, KT, HPC * HD], BF16,
                         kind="ExternalInput")
    wvd = nc.dram_tensor("wv_t", [128, KT, HPC * HD], BF16,
                         kind="ExternalInput")
    cosT = nc.dram_tensor("cosT", [HD, S], BF16, kind="ExternalInput")
    sinT = nc.dram_tensor("sinT", [HD, S], BF16, kind="ExternalInput")
    maskd = nc.dram_tensor("mask", [128, 128], BF16, kind="ExternalInput")
    # laid out exactly as the VMM's o_sb tiles ([ql, qt, dl] per span chunk)
    # so the output DMA is a straight contiguous dump; the host transpose
    # absorbs the permutation for free.
    attnout = nc.dram_tensor(
        "attnout", [B, HPC, NSPANS, 128, 4, 128], BF16, kind="ExternalOutput"
    )

    with tile.TileContext(nc) as tc:
        with (
            tc.tile_pool(name="persist", bufs=1) as persist,
            tc.tile_pool(name="xpool", bufs=3) as xpool,
            tc.tile_pool(name="rope", bufs=2) as rope,
            tc.tile_pool(name="epool", bufs=17) as epool,
            tc.tile_pool(name="opool", bufs=2) as opool,
            tc.tile_pool(name="rpool", bufs=4) as rpool,
            tc.tile_pool(name="ps_qk", bufs=2, space="PSUM") as ps_qk,
            tc.tile_pool(name="ps_sc", bufs=2, space="PSUM") as ps_sc,
            tc.tile_pool(name="ps_o", bufs=2, space="PSUM") as ps_o,
        ):
            # ---------------- persistent tiles ----------------
            wq_bf = persist.tile([128, KT, HPC * HD], BF16, tag="wq_bf")
            wk_bf = persist.tile([128, KT, HPC * HD], BF16, tag="wk_bf")
            wv_bf = persist.tile([128, KT, HPC * HD], BF16, tag="wv_bf")
            cos_sb = persist.tile([HD, S], BF16, tag="cos_sb")
            sin_sb = persist.tile([HD, S], BF16, tag="sin_sb")
            mask_bf = persist.tile([128, 128], BF16, tag="mask_bf")
            q_sb = persist.tile([HD, B, HPC, S], BF16, tag="q_sb")
            k_sb = persist.tile([HD, B, HPC, S], BF16, tag="k_sb")
            # v with an appended ones column (denominator trick)
            v_sb = persist.tile([128, B, TT, HPC, HD + 1], BF16, tag="v_sb")

            def load_xspan(b, span, pieces=1):
                xspan = xpool.tile([128, KT, SPAN], BF16, tag="x")
                if pieces == 1:
                    nc.sync.dma_start(xspan[:], xd[b, span])
                else:
                    step = KT // pieces
                    for p in range(pieces):
                        nc.sync.dma_start(
                            xspan[:, p * step:(p + 1) * step, :],
                            xd[b, span, :, p * step:(p + 1) * step, :],
                        )
                return xspan

            def qkv_qk(b, span, xspan):
                sl = slice(span * SPAN, (span + 1) * SPAN)
                for h in range(HPC):
                    hsl = slice(h * HD, (h + 1) * HD)
                    for wbf, dst in ((wq_bf, q_sb), (wk_bf, k_sb)):
                        ps = ps_qk.tile([128, SPAN], F32, tag="qk")
                        for kt in range(KT):
                            nc.tensor.matmul(
                                ps[:],
                                wbf[:, kt, hsl],
                                xspan[:, kt, :],
                                start=(kt == 0),
                                stop=(kt == KT - 1),
                            )
                        # RoPE: out = p*cos + rot(p)*sin_signed
                        pf = rope.tile([128, SPAN], F32, tag="pf")
                        nc.scalar.copy(pf[:], ps[:])
                        rot = rope.tile([128, SPAN], F32, tag="rot")
                        nc.sync.dma_start(rot[0:64, :], pf[64:128, :])
                        nc.sync.dma_start(rot[64:128, :], pf[0:64, :])
                        nc.vector.tensor_mul(pf[:], pf[:], cos_sb[:, sl])
                        nc.vector.tensor_mul(rot[:], rot[:], sin_sb[:, sl])
                        nc.vector.tensor_add(dst[:, b, h, sl], pf[:], rot[:])

            def qkv_v(b, span, xspan):
                # V projection (natural [token, head*hd] layout)
                for j in range(4):
                    tt = span * 4 + j
                    psv = ps_qk.tile([128, HPC * HD], F32, tag="qk")
                    for kt in range(KT):
                        nc.tensor.matmul(
                            psv[:],
                            xspan[:, kt, j * 128:(j + 1) * 128],
                            wv_bf[:, kt, :],
                            start=(kt == 0),
                            stop=(kt == KT - 1),
                        )
                    for h in range(HPC):
                        nc.vector.tensor_copy(
                            v_sb[:, b, tt, h, 0:HD],
                            psv[:, h * HD:(h + 1) * HD],
                        )

            def qkv_span(b, span, xspan):
                qkv_qk(b, span, xspan)
                qkv_v(b, span, xspan)

            def attn_scores(b, h, s):
                # causal: k tiles 0 .. 4s+3; diagonal k-tiles (jd = kt-4s >= 0)
                # only need queries q >= jd*128, so both the score matmul and
                # the exp are trimmed to the valid sub-range.
                nkt = 4 * s + 4
                es = []
                for kp in range(nkt // 2):
                    # two k-tiles share a 2-bank PSUM: one ACTIVATE covers both
                    # exps when untrimmed, amortizing ScalarE's per-op overhead
                    psc = ps_sc.tile([128, 2 * SPAN], F32, tag="sc")
                    q0s = []
                    for half in range(2):
                        kt = 2 * kp + half
                        jd = kt - 4 * s
                        q0 = max(jd, 0) * 128
                        q0s.append(q0)
                        nc.tensor.matmul(
                            psc[:, half * SPAN + q0:(half + 1) * SPAN],
                            k_sb[:, b, h, kt * 128:(kt + 1) * 128],
                            q_sb[:, b, h, s * SPAN + q0:(s + 1) * SPAN],
                            start=True,
                            stop=True,
                        )
                    e2 = epool.tile([128, 2 * SPAN], BF16, tag="e")
                    # ACTIVATE costs (N+352)/1.2 ns: splitting the pair into
                    # two ops only pays when half1 skips >352 columns. The
                    # skipped-over PSUM columns were never written this group;
                    # exp of that stale data lands in e2 columns no attn@V
                    # matmul ever reads.
                    if q0s[1] > 352:
                        for half in range(2):
                            q0 = q0s[half]
                            nc.scalar.activation(
                                e2[:, half * SPAN + q0:(half + 1) * SPAN],
                                psc[:, half * SPAN + q0:(half + 1) * SPAN],
                                mybir.ActivationFunctionType.Exp, scale=SCALE,
                            )
                    else:
                        nc.scalar.activation(
                            e2[:, q0s[0]:], psc[:, q0s[0]:],
                            mybir.ActivationFunctionType.Exp, scale=SCALE
                        )
                    for half in range(2):
                        kt = 2 * kp + half
                        jd = kt - 4 * s
                        base = half * SPAN
                        if jd >= 0:  # diagonal block: zero out k > q
                            nc.vector.tensor_mul(
                                e2[:, base + jd * 128:base + (jd + 1) * 128],
                                e2[:, base + jd * 128:base + (jd + 1) * 128],
                                mask_bf[:],
                            )
                        es.append(e2[:, base:base + SPAN])
                return es

            def attn_vmm(b, h, s, es, final=False):
                o_sb = opool.tile([128, 4, 128], BF16, tag="o")
                for j in range(4):
                    last_kt = 4 * s + j
                    pso = ps_o.tile([128, HD + 1], F32, tag="o")
                    for kt in range(last_kt + 1):
                        nc.tensor.matmul(
                            pso[:],
                            es[kt][:, j * 128:(j + 1) * 128],
                            v_sb[:, b, kt, h, :],
                            start=(kt == 0),
                            stop=(kt == last_kt),
                        )
                    recip = rpool.tile([128, 1], F32, tag="recip")
                    nc.vector.reciprocal(recip[:], pso[:, HD:HD + 1])
                    nc.vector.tensor_scalar_mul(
                        o_sb[:, j, :], pso[:, 0:HD], recip[:]
                    )
                    if final and j == 1:
                        # the very last chunk's output DMA sits in the kernel
                        # tail: stream the first half while j=2,3 compute
                        nc.sync.dma_start(
                            attnout[b, h, s, :, 0:2, :], o_sb[:, 0:2, :])
                if final:
                    nc.sync.dma_start(
                        attnout[b, h, s, :, 2:4, :], o_sb[:, 2:4, :])
                else:
                    nc.sync.dma_start(attnout[b, h, s], o_sb[:])

            # ---------------- emission schedule ----------------
            # DMA pushes go out in emission order and the engines round-robin
            # between queues, so the loads gating the earliest matmuls are
            # split into pieces and interleaved need-first: wq/x kt-quarters
            # alternating, then cos/sin (RoPE), then wk/wv quarters.
            step = KT // 4

            def wpiece(wbf, wdram, p):
                nc.sync.dma_start(wbf[:, p * step:(p + 1) * step, :],
                                  wdram[:, p * step:(p + 1) * step, :])

            xspan0 = xpool.tile([128, KT, SPAN], BF16, tag="x")

            def xpiece(p):
                nc.sync.dma_start(xspan0[:, p * step:(p + 1) * step, :],
                                  xd[0, 0, :, p * step:(p + 1) * step, :])

            # need-by times (first-MM ~12us): wq/x pieces pace the Q(h0)
            # group through ~16us, wk by ~19, wv by ~27; cos/sin only feed
            # the DVE RoPE chain whose output PE reads at ~30, mask at ~34.
            wpiece(wq_bf, wqd, 0)
            xpiece(0)
            wpiece(wq_bf, wqd, 1)
            xpiece(1)
            wpiece(wq_bf, wqd, 2)
            xpiece(2)
            wpiece(wq_bf, wqd, 3)
            xpiece(3)
            for p in range(4):
                wpiece(wk_bf, wkd, p)
            nc.sync.dma_start(cos_sb[:], cosT[:])
            nc.sync.dma_start(sin_sb[:], sinT[:])
            for p in range(4):
                wpiece(wv_bf, wvd, p)
            nc.sync.dma_start(mask_bf[:], maskd[:])
            nc.vector.memset(v_sb[:, :, :, :, HD], 1.0)

            # HAM warmup: >3.4us of dummy matmuls while the first loads land,
            # so the real matmul stream starts at the full 2.4 GHz clock.
            warm = persist.tile([128, 128], BF16, tag="warm")
            nc.vector.memset(warm[:], 0.0)
            psw = ps_qk.tile([128, 128], F32, tag="qk")
            for i in range(60):
                nc.tensor.matmul(psw[:], warm[:], warm[:],
                                 start=(i == 0), stop=(i == 59))
            x00 = xspan0

            # chunk order: batch 0 ascending spans, batch 1 descending spans
            # (the final chunks are the smallest -> short kernel tail).
            chunks = (
                [(0, h, s) for s in range(NSPANS) for h in range(HPC)]
                + [(1, h, s) for s in reversed(range(NSPANS))
                   for h in range(HPC)]
            )
            fillers = [(0, 1), (0, 2), (0, 3), (1, 0), (1, 1), (1, 2), (1, 3)]

            qkv_span(0, 0, x00)
            es_map = {0: attn_scores(*chunks[0])}

            # Software-pipeline: between a chunk's score matmuls (exps on
            # ScalarE) and its attn@V matmuls (consuming those exps on
            # TensorE), emit independent TensorE work — one QKV span of
            # filler — so the exp latency hides behind matmuls.
            last = len(chunks) - 1
            for i in range(len(chunks)):
                if i < len(fillers):
                    fb, fs = fillers[i]
                    qkv_span(fb, fs, load_xspan(fb, fs))
                if i + 1 < len(chunks) and i + 1 not in es_map:
                    es_map[i + 1] = attn_scores(*chunks[i + 1])
                if i + 2 == last:
                    # the final chunk's exp needs more PE work in front of it
                    # than the small second-to-last VMM provides: emit its
                    # scores one iteration early so ScalarE runs ahead.
                    es_map[last] = attn_scores(*chunks[last])
                attn_vmm(*chunks[i], es_map.pop(i), final=(i == last))
    return nc


# ---------------------------------------------------------------------------
# NEFF 2: output projection, token-parallel
# ---------------------------------------------------------------------------

def build_oproj_nc():
    """out[tok, hout] on a 2x4 (token-half x hout-quarter) core grid.
    Token-major loop: each 128-token tile runs its full K-contiguous
    accumulation, so per-tile flushes overlap the next tile's matmuls and
    the a/wo streams hide under compute."""
    nc = bass.Bass(target_bir_lowering=False, debug=False)

    ad = nc.dram_tensor("a_t", [T_TILES, 128, KT, 128], BF16,
                        kind="ExternalInput")
    wod = nc.dram_tensor("wo_t", [128, KT, HOUT], BF16, kind="ExternalInput")
    out = nc.dram_tensor("out", [T_TILES * 128, HOUT], F32,
                         kind="ExternalOutput")

    with tile.TileContext(nc) as tc:
        with (
            tc.tile_pool(name="persist", bufs=1) as persist,
            tc.tile_pool(name="apool", bufs=3) as apool,
            tc.tile_pool(name="outp", bufs=3) as outp,
            tc.tile_pool(name="psum", bufs=3, space="PSUM") as psum,
        ):
            wo_bf = persist.tile([128, KT, HOUT], BF16, tag="wo_bf")

            def load_a(m):
                t = apool.tile([128, KT, 128], BF16, tag="a")
                nc.sync.dma_start(t[:], ad[m])
                return t

            # need-first push order: tile 0's first k-tiles and wo's first
            # chunk gate the first matmul; interleave them ahead of the rest.
            def wo_chunk(c):
                nc.sync.dma_start(
                    wo_bf[:, c * 4:(c + 1) * 4, :],
                    wod[:, c * 4:(c + 1) * 4, :],
                )

            a0 = apool.tile([128, KT, 128], BF16, tag="a")
            nc.sync.dma_start(a0[:, 0:8, :], ad[0, :, 0:8, :])
            wo_chunk(0)
            nc.sync.dma_start(a0[:, 8:16, :], ad[0, :, 8:16, :])
            wo_chunk(1)
            a_tiles = {0: a0, 1: load_a(1)}
            wo_chunk(2)
            wo_chunk(3)

            # HAM warmup: >3.4us of dummy matmuls while the first loads land
            warm = persist.tile([128, 128], BF16, tag="warm")
            nc.vector.memset(warm[:], 0.0)
            psw = psum.tile([128, 128], F32, tag="ps")
            for i in range(64):
                nc.tensor.matmul(psw[:], warm[:], warm[:],
                                 start=(i == 0), stop=(i == 63))

            for m in range(T_TILES):
                if m + 2 < T_TILES:
                    a_tiles[m + 2] = load_a(m + 2)
                ps = psum.tile([128, HOUT], F32, tag="ps")
                at = a_tiles.pop(m)
                for kt in range(KT):
                    nc.tensor.matmul(
                        ps[:],
                        at[:, kt, :],
                        wo_bf[:, kt, :],
                        start=(kt == 0),
                        stop=(kt == KT - 1),
                    )
                o = outp.tile([128, HOUT], F32, tag="o")
                nc.scalar.copy(o[:], ps[:])
                if m == T_TILES - 1:
                    # final flush sits in the kernel tail: halve its
                    # completion latency with two parallel queue pushes
                    half = HOUT // 2
                    nc.sync.dma_start(
                        out[m * 128:(m + 1) * 128, 0:half], o[:, 0:half])
                    nc.sync.dma_start(
                        out[m * 128:(m + 1) * 128, half:], o[:, half:])
                else:
                    nc.sync.dma_start(out[m * 128:(m + 1) * 128, :], o[:])
    return nc


# ---------------------------------------------------------------------------
# host driver
# ---------------------------------------------------------------------------

_NC_CACHE = {}


def _get_ncs():
    if "attn" not in _NC_CACHE:
        _NC_CACHE["attn"] = build_attn_nc()
        _NC_CACHE["oproj"] = build_oproj_nc()
    return _NC_CACHE["attn"], _NC_CACHE["oproj"]


def _rope_tables():
    inv_freq = 1.0 / (10000.0 ** (np.arange(0, HD, 2, dtype=np.float32) / HD))
    t = np.arange(S, dtype=np.float32)
    freqs = np.einsum("i,j->ij", t, inv_freq)  # [S, HD/2]
    emb = np.concatenate([freqs, freqs], axis=-1)  # [S, HD]
    cos = np.cos(emb).astype(np.float32)
    sin = np.sin(emb).astype(np.float32)
    cosT = np.ascontiguousarray(cos.T)  # [HD, S]
    sinT = np.ascontiguousarray(sin.T)
    sinT_signed = sinT.copy()
    sinT_signed[0:64, :] *= -1.0  # fold rotate_half's negation into the table
    return cosT, sinT_signed


def _tile_w(block):
    """[256 rows, HIDDEN] weight block -> bf16 [128, KT, 256] (hid-tiled)."""
    return np.ascontiguousarray(
        block.T.reshape(KT, 128, HPC * HD).transpose(1, 0, 2)
    ).astype(NP_BF16)


def kernel(x, Wq, Wk, Wv, Wo):
    x = np.asarray(x, dtype=np.float32)
    Wq = np.asarray(Wq, dtype=np.float32)
    Wk = np.asarray(Wk, dtype=np.float32)
    Wv = np.asarray(Wv, dtype=np.float32)
    Wo = np.asarray(Wo, dtype=np.float32)

    nc1, nc2 = _get_ncs()
    core_ids = list(range(N_CORES))
    trace = bool(os.environ.get("BASS_TRACE"))

    cosT, sinT_signed = _rope_tables()
    mask = np.triu(np.ones((128, 128), dtype=np.float32)).astype(NP_BF16)
    # x tiled: [B, span, p, ko, t] with 16KB contiguous per-partition lines
    xt = np.ascontiguousarray(
        x.reshape(B, NSPANS, SPAN, KT, 128).transpose(0, 1, 4, 3, 2)
    ).astype(NP_BF16)

    in_maps1 = []
    for c in range(N_CORES):
        csl = slice(c * HPC * HD, (c + 1) * HPC * HD)
        in_maps1.append(
            {
                "x_t": xt,
                "wq_t": _tile_w(Wq[csl, :]),
                "wk_t": _tile_w(Wk[csl, :]),
                "wv_t": _tile_w(Wv[csl, :]),
                "cosT": cosT.astype(NP_BF16),
                "sinT": sinT_signed.astype(NP_BF16),
                "mask": mask,
            }
        )

    LAST_RESULTS.clear()
    res1 = run_bass_kernel_spmd(nc1, in_maps1, core_ids=core_ids, trace=trace)
    LAST_RESULTS.append(res1)

    # host relayout: per-head attention outputs -> per-core NEFF2 inputs
    # arr axes (co, b, h, s, ql, qt, dl);  hid = co*256 + h*128 + dl,
    # tok = b*2048 + (s*4 + qt)*128 + ql
    arr = np.stack([res1.results[c]["attnout"] for c in range(N_CORES)])
    # a_full[ti][tile, p, kt(head), tok] with 4KB contiguous partition lines
    a_full = np.ascontiguousarray(
        arr.transpose(1, 3, 5, 6, 0, 2, 4)
    ).reshape(B, T_TILES, 128, KT, 128)

    woT = Wo.T  # [hid, hout]
    in_maps2 = []
    for c in range(N_CORES):
        ti, hj = c // 4, c % 4
        wo_t = np.ascontiguousarray(
            woT[:, hj * HOUT:(hj + 1) * HOUT]
            .reshape(KT, 128, HOUT).transpose(1, 0, 2)
        ).astype(NP_BF16)
        in_maps2.append({"a_t": a_full[ti], "wo_t": wo_t})
    res2 = run_bass_kernel_spmd(nc2, in_maps2, core_ids=core_ids, trace=trace)
    LAST_RESULTS.append(res2)

    TOKS = T_TILES * 128
    out = np.empty((B * S, HIDDEN), dtype=np.float32)
    for c in range(N_CORES):
        ti, hj = c // 4, c % 4
        out[ti * TOKS:(ti + 1) * TOKS, hj * HOUT:(hj + 1) * HOUT] = (
            res2.results[c]["out"]
        )
    return np.ascontiguousarray(out.reshape(B, S, HIDDEN), dtype=np.float32)
